# revision 5
# baseline (speedup 1.0000x reference)
# Multi-head self-attention kernel for Trainium2, 8 NeuronCores — v5.
# Sharding: data-parallel over batch (b=8 -> one batch per core).
#
# v2 over baseline:
#   - fp16 host-side inputs (hsT/wq/wk/wv/bv): halves DMA + SBUF footprint.
#   - software-pipelined emission: scores of pair p+2 interleave with PV of
#     pair p so the PE never waits on the scalar-engine exps; exp supply
#     starts ~15us into the kernel instead of after all projections.
#   - v-bias folded into the V-projection evacuation (PV ones-column then
#     yields out = softmax@v + bv directly; no per-head bias add).
#   - psum: 2 slot tags x 2 bufs x 2 banks = 8 banks exactly.
#
# Math per core (batch b), hsT = hs[b].T [E, L]:
#   qT[e,l], kT[e,l] = w @ hsT (+bias);  v[l,e] = hsT.T @ wv  (+bv folded)
#   scoresT[lk,lq] per head (K=64; head pairs at partitions 0-63/64-127
#     auto row-tile via tile_position inference)
#   expT = exp(scoresT/8) fp16;  pv[lq,0:65] = expT.T @ [v_h+bv | 1]
#   out_h[lq,d] = pv[lq,d] / pv[lq,64]
import contextlib

import numpy as np

B, L, E = 8, 1024, 1024
H, D = 16, 64
NC = 8          # cores
P = 128         # partitions
CH = E // P     # 8 contraction chunks
MT = E // P     # 8 e_out tiles
LT = L // P     # 8 l tiles
HPT = P // D    # 2 heads per tile-pair
VW = D + 2      # 66: v cols per head (64 + ones col + pad)

TRACE = False
LOOP_K = 1      # >1: wrap body in a HW loop (benchmarking only)
ABLATE = set()  # timing ablations: "noexp", "nopv", "noqk", "nov"
_cached = {}


def _build():
    import concourse.bacc as bacc
    import concourse.mybir as mybir
    import concourse.tile as tile
    import concourse.bass as bass

    F32 = mybir.dt.float32
    F16 = mybir.dt.float16
    Exp = mybir.ActivationFunctionType.Exp

    nc = bacc.Bacc("TRN2", target_bir_lowering=False, debug=False)
    hsT = nc.dram_tensor("hsT", [E, L], F16, kind="ExternalInput").ap()
    wqT = nc.dram_tensor("wqT", [E, E], F16, kind="ExternalInput").ap()
    wkT = nc.dram_tensor("wkT", [E, E], F16, kind="ExternalInput").ap()
    wvT = nc.dram_tensor("wvT", [E, E], F16, kind="ExternalInput").ap()
    bq = nc.dram_tensor("bq", [E], F32, kind="ExternalInput").ap()
    bk = nc.dram_tensor("bk", [E], F32, kind="ExternalInput").ap()
    bv = nc.dram_tensor("bv", [E], F16, kind="ExternalInput").ap()
    out = nc.dram_tensor("out", [H, L, D], F32, kind="ExternalOutput").ap()

    with tile.TileContext(nc) as tc:
        with tc.tile_pool(name="big", bufs=1) as big, \
             tc.tile_pool(name="wpool", bufs=2) as wpool, \
             tc.tile_pool(name="epool", bufs=56) as epool, \
             tc.tile_pool(name="spool", bufs=2) as spool, \
             tc.tile_pool(name="psum", bufs=2, space="PSUM") as pp, \
             (tc.For_i(0, LOOP_K, 1) if LOOP_K > 1
              else contextlib.nullcontext()):

            # ---- constants / biases / resident tensors ----
            bq_sb = big.tile([P, MT], F32)
            bk_sb = big.tile([P, MT], F32)
            nc.sync.dma_start(out=bq_sb, in_=bq.rearrange("(m p) -> p m", p=P))
            nc.sync.dma_start(out=bk_sb, in_=bk.rearrange("(m p) -> p m", p=P))
            bv_bc = big.tile([P, E], F16)
            nc.sync.dma_start(
                out=bv_bc,
                in_=bass.AP(tensor=bv.tensor, offset=0, ap=[[0, P], [1, E]]),
            )
            bv4 = bv_bc.rearrange("p (h c) -> p h c", h=H)

            hsT_sb = big.tile([P, CH, L], F16)
            for c in range(CH):
                nc.sync.dma_start(out=hsT_sb[:, c, :], in_=hsT[c * P:(c + 1) * P, :])
            wv_sb = big.tile([P, CH, E], F16)
            for c in range(CH):
                nc.sync.dma_start(out=wv_sb[:, c, :], in_=wvT[c * P:(c + 1) * P, :])

            qT_sb = big.tile([P, MT, L], F16)
            kT_sb = big.tile([P, MT, L], F16)
            v_sb = big.tile([P, LT, H * VW], F16)
            v4 = v_sb.rearrange("p m (h c) -> p m h c", h=H)
            nc.vector.memset(v4[:, :, :, D:VW], 1.0)

            exps = [[[None] * CH for _ in range(HPT)] for _ in range(MT)]
            pvs = [[None] * HPT for _ in range(MT)]

            # ---------------- work units ----------------
            def qk_unit(m):
                for (wT, dst, bias, nm) in ((wqT, qT_sb, bq_sb, "q"),
                                            (wkT, kT_sb, bk_sb, "k")):
                    wt = wpool.tile([P, CH, P], F16, tag="w", name=f"w{nm}{m}")
                    nc.sync.dma_start(
                        out=wt,
                        in_=wT[:, m * P:(m + 1) * P].rearrange(
                            "(c p) n -> p c n", p=P),
                    )
                    ps = pp.tile([P, 1024], F32, tag="mm", name=f"ps{nm}{m}")
                    for c in range(CH):
                        for n in range(2):
                            nc.tensor.matmul(
                                ps[:, n * 512:(n + 1) * 512],
                                wt[:, c, :],
                                hsT_sb[:, c, n * 512:(n + 1) * 512],
                                start=(c == 0), stop=(c == CH - 1),
                            )
                    nc.vector.tensor_scalar_add(dst[:, m, :], ps, bias[:, m:m + 1])

            def v_unit(m):
                ps = pp.tile([P, 1024], F32, tag="mm", name=f"psv{m}")
                for c in range(CH):
                    for n in range(2):
                        nc.tensor.matmul(
                            ps[:, n * 512:(n + 1) * 512],
                            hsT_sb[:, c, m * P:(m + 1) * P],
                            wv_sb[:, c, n * 512:(n + 1) * 512],
                            start=(c == 0), stop=(c == CH - 1),
                        )
                nc.vector.tensor_add(
                    v4[:, m, :, 0:D],
                    ps.rearrange("p (h c) -> p h c", h=H),
                    bv4,
                )

            def sc_unit(p_i, lk):
                # both head-halves, matmuls emitted alternating so the
                # auto-row-tiled 64-partition pairs overlap in the PE array
                scs, es = [], []
                for half in range(HPT):
                    scs.append(pp.tile([P, 1024], F32, tag="sc",
                                       name=f"sc{p_i}_{lk}_{half}"))
                    es.append(epool.tile([P, 1024], F16, tag="e",
                                         name=f"e{p_i}_{lk}_{half}"))
                for n in range(2):
                    for half in range(HPT):
                        lo = half * D
                        nc.tensor.matmul(
                            scs[half][:, n * 512:(n + 1) * 512],
                            kT_sb[lo:lo + D, p_i, lk * P:(lk + 1) * P],
                            qT_sb[lo:lo + D, p_i, n * 512:(n + 1) * 512],
                            start=True, stop=True,
                        )
                for half in range(HPT):
                    if "noexp" not in ABLATE:
                        nc.scalar.activation(es[half], scs[half], Exp,
                                             scale=0.125)
                    exps[p_i][half][lk] = es[half]

            def pv_unit(p_i, half, t):
                if t == 0:
                    pvs[p_i][half] = pp.tile([P, LT, P], F32, tag="mm",
                                             name=f"pv{p_i}_{half}")
                pv = pvs[p_i][half]
                h = HPT * p_i + half
                for c in range(LT):
                    nc.tensor.matmul(
                        pv[:, t, 0:D + 1],
                        exps[p_i][half][c][:, t * P:(t + 1) * P],
                        v_sb[:, c, h * VW:h * VW + D + 1],
                        start=(c == 0), stop=(c == LT - 1),
                    )

            def norm_unit(p_i, half):
                h = HPT * p_i + half
                pv = pvs[p_i][half]
                rs = spool.tile([P, LT], F32, tag="rs", name=f"rs{p_i}_{half}")
                nc.vector.reciprocal(rs, pv[:, :, D:D + 1].squeeze(2))
                st = spool.tile([P, LT, D], F32, tag="st",
                                name=f"st{p_i}_{half}")
                for t in range(LT):
                    nc.vector.tensor_scalar_mul(
                        st[:, t, :], pv[:, t, 0:D], rs[:, t:t + 1])
                nc.sync.dma_start(
                    out=out[h].rearrange("(t p) d -> p t d", p=P),
                    in_=st,
                )

            # ---------------- emission schedule ----------------
            def sc_pair(p_i):
                return [(sc_unit, (p_i, lk)) for lk in range(LT)]

            def pv_pair(p_i):
                us = []
                for hf in range(HPT):
                    us += [(pv_unit, (p_i, hf, t)) for t in range(LT)]
                    us.append((norm_unit, (p_i, hf)))
                return us

            def run(units):
                for f, a in units:
                    f(*a)

            def mix(a_units, b_units):
                # proportional round-robin merge preserving each list's order
                na, nb = len(a_units), len(b_units)
                out_u, ia, ib = [], 0, 0
                for k in range(na + nb):
                    if ia * (nb or 1) * 1.0 <= ib * (na or 1) and ia < na:
                        out_u.append(a_units[ia]); ia += 1
                    elif ib < nb:
                        out_u.append(b_units[ib]); ib += 1
                    else:
                        out_u.append(a_units[ia]); ia += 1
                return out_u

            if "nov" in ABLATE:
                v_units = []
                nc.vector.memset(v_sb, 0.5)
            else:
                v_units = [(v_unit, (m,)) for m in range(MT)]
            qk_unit(0)
            qk_unit(1)
            # pair 0 scores interleave with V projection (Act starts early)
            run(mix(sc_pair(0), v_units))
            # pair 1 scores interleave with remaining Q/K projections
            if "noqk" not in ABLATE:
                run(mix(sc_pair(1), [(qk_unit, (m,)) for m in range(2, MT)]))
            else:
                run(sc_pair(1))
            # steady state: PV of pair p with scores of pair p+2
            for p_i in range(MT):
                sc_next = sc_pair(p_i + 2) if p_i + 2 < MT else []
                if "nopv" in ABLATE:
                    run(sc_next)
                else:
                    run(mix(pv_pair(p_i), sc_next))

    nc.compile()
    return nc


def _get_nc():
    if "nc" not in _cached:
        _cached["nc"] = _build()
    return _cached["nc"]


def make_in_maps(inputs):
    hs = np.asarray(inputs["hidden_states"], dtype=np.float32)
    hsT = np.ascontiguousarray(hs.transpose(0, 2, 1)).astype(np.float16)
    wqT = np.ascontiguousarray(
        np.asarray(inputs["w_q"], np.float32).T).astype(np.float16)
    wkT = np.ascontiguousarray(
        np.asarray(inputs["w_k"], np.float32).T).astype(np.float16)
    wvT = np.ascontiguousarray(
        np.asarray(inputs["w_v"], np.float32).T).astype(np.float16)
    return [
        {"hsT": hsT[i], "wqT": wqT, "wkT": wkT, "wvT": wvT,
         "bq": np.asarray(inputs["b_q"], np.float32),
         "bk": np.asarray(inputs["b_k"], np.float32),
         "bv": np.asarray(inputs["b_v"], np.float16)}
        for i in range(NC)
    ]


def kernel(hidden_states, w_q, b_q, w_k, b_k, w_v, b_v):
    from concourse import bass_utils

    nc = _get_nc()
    in_maps = make_in_maps(dict(
        hidden_states=hidden_states, w_q=w_q, b_q=b_q, w_k=w_k, b_k=b_k,
        w_v=w_v, b_v=b_v))
    res = bass_utils.run_bass_kernel_spmd(
        nc, in_maps, core_ids=list(range(NC)), trace=TRACE)
    kernel.last_exec_time_ns = res.exec_time_ns
    kernel.last_results = res.results
    return np.stack([res.results[i]["out"].reshape(L, H * D) for i in range(NC)])


kernel.last_exec_time_ns = None
